# revision 65
# baseline (speedup 1.0000x reference)
"""Trainium2 Bass kernel v3: PINN MLP forward + JVP derivs (T, Tz, Tt, Tzz).

Math per point (feature-major), layer l: h = tanh(a), g = 1 - h^2:
  forward:      a_{l+1} = W^T h_l
  1st deriv:    a'_{l+1} = W^T (g_l * a'_l)         (z and t directions)
  2nd deriv(z): a''_{l+1} = W^T (g_l * (a''_l - 2 h_l a'_l^2))
L1 derivative seeds fold into host-precomputed Z2/T2/ZZ2; the z-chain
carries sqrt(2) (via Z2 and the W4z column) so 2*az^2 is a plain Square.

v3 vs the 453us v2 baseline (this version: ~272us):
  - ALL matmuls fp16 including L1 (x cast to fp16 host-side); PE runs
    warm at the 213ns/512-col stream rate.
  - g is never materialized past L1: every (g*x) product is a fused
    scalar_tensor_tensor (hh - 1) * x on the DVE reading the derivative
    preactivations DIRECTLY from PSUM -- the baseline's ACT az|at copies
    and GPSIMD fp16 pair-muls are gone.
  - Sign bookkeeping makes the products come out positive at L2, so W3
    is shared by the forward/z/t chains at L3 (only papp3 needs -W3).
  - sq = 2*az'^2 comes from ACT Square reading PSUM.
  - SBUF elementwise runs at width 1024+ (fp16 TT/TS at width 512 hits
    a ~2x DVE slow path on this silicon); PSUM-source STTs use a
    stride-0 broadcast AP on in0 to cover az|at in one op.
  - Const weights ship as one packed [128,800] DMA.
  - Measured engine busy: DVE ~85% (bottleneck), GPSIMD/PE ~80%, ACT ~70%.

Engine split per pair (2 x 512 points):
  PE:     25 matmul streams (L1 2, L2 8, negI 3, L3 8, L4 4)
  ACT:    tanh x5, Square x4 (sq2 a/b, sq3, hh3), mg1 a-half as
          Copy(hh1, bias=-1), p4 copy
  DVE:    mg1 b-half TS, u1n/hh2 TT-1024, ztw STT-1024(PSUM) x2,
          mhpp2 STT x2, zt3 STT-1024(PSUM), mhpp3 STT
  GPSIMD: hh1 x2, v2 x2, v3 (SBUF fp16 narrow muls)

Sharding: pure data parallel, 262144 points -> 8 cores x 32768.
"""

import sys

import numpy as np

sys.path.insert(0, "/opt/trn_rl_repo")

import concourse.bass as bass
import concourse.bacc as bacc
import concourse.tile as tile
from concourse import mybir
from concourse.bass_utils import run_bass_kernel_spmd

N = 262144
NCORES = 8
NSH = N // NCORES  # 32768 points per core
B = 512  # points per tile
NT = NSH // B  # 64 tiles
NP = NT // 2  # 32 tile pairs
CH = 4096  # x chunk (points) per input DMA
TPC = CH // B  # tiles per chunk

F32 = mybir.dt.float32
F16 = mybir.dt.float16

TRACE = False
LAST_RESULT = None


def _build():
    nc = bacc.Bacc(None, target_bir_lowering=False)

    xT = nc.declare_dram_parameter("xT", [3, NSH], F16, isOutput=False)
    W1 = nc.declare_dram_parameter("W1", [3, 128], F16, isOutput=False)
    # packed: W2|Z2|T2|ZZ2|W3|W3n|W4h|W4z|W4t|W4p|negI  (128 x 800 fp16)
    Wpk = nc.declare_dram_parameter("Wpk", [128, 800], F16, isOutput=False)
    bpk = nc.declare_dram_parameter("bpk", [128, 4], F32, isOutput=False)
    out_d = nc.declare_dram_parameter("out", [4, NSH], F32, isOutput=True)

    Tanh = mybir.ActivationFunctionType.Tanh
    Square = mybir.ActivationFunctionType.Square
    mult = mybir.AluOpType.mult
    sub = mybir.AluOpType.subtract

    def stt(eng, out, in0, in1):
        # out = (in0 - 1) * in1
        eng.scalar_tensor_tensor(
            out=out, in0=in0, scalar=1.0, in1=in1, op0=sub, op1=mult
        )

    def bc2(t_ap):
        """[128,512] AP -> broadcast [128, 2x512] (block repeated twice)."""
        return bass.AP(
            tensor=t_ap.tensor,
            offset=t_ap.offset,
            ap=[t_ap.ap[0], [0, 2], [1, B]],
        )

    def split2(t_ap, half_stride):
        """[128, 2*B] write AP whose halves land `half_stride` cols apart."""
        return bass.AP(
            tensor=t_ap.tensor,
            offset=t_ap.offset,
            ap=[t_ap.ap[0], [half_stride, 2], [1, B]],
        )

    with tile.TileContext(nc) as tc:
        with (
            tc.tile_pool(name="consts", bufs=1) as consts,
            tc.tile_pool(name="xin", bufs=2) as xin,
            tc.tile_pool(name="l1", bufs=4) as l1p,
            tc.tile_pool(name="l2", bufs=4) as l2p,
            tc.tile_pool(name="l3", bufs=4) as l3p,
            tc.tile_pool(name="sb4", bufs=4) as sb4p,
            tc.tile_pool(name="paP", bufs=2, space="PSUM") as paP,
            tc.tile_pool(name="dztP", bufs=2, space="PSUM") as dztP,
            tc.tile_pool(name="appP", bufs=2, space="PSUM") as appP,
        ):
            W1s = consts.tile([3, 128], F16)
            nc.sync.dma_start(out=W1s, in_=W1[:])
            Wp = consts.tile([128, 800], F16)
            nc.sync.dma_start(out=Wp, in_=Wpk[:])
            bp = consts.tile([128, 4], F32)
            nc.sync.dma_start(out=bp, in_=bpk[:])
            W2s = Wp[:, 0:128]
            Z2s = Wp[:, 128:256]
            T2s = Wp[:, 256:384]
            ZZ2s = Wp[:, 384:512]
            W3s = Wp[:, 512:576]
            W3ns = Wp[:, 576:640]
            W4hs = Wp[:, 640:648]
            W4zs = Wp[:, 648:656]
            W4ts = Wp[:, 656:664]
            W4ps = Wp[:, 664:672]
            negIs = Wp[:, 672:800]
            b1s = bp[:, 0:1]
            b2s = bp[:, 1:2]
            b3ds = bp[:, 2:3]
            bneg1 = bp[:, 3:4]

            xcs = {}

            def stage1(p):
                """L1 + L2 for pair p. Returns live tiles for stage2."""
                ta = 2 * p
                c = ta // TPC
                ci = ta % TPC
                if ci == 0:
                    with tc.high_priority(offset=120):
                        xcs[c] = xin.tile([3, CH], F16, tag="xc", name=f"xc{c}")
                        nc.sync.dma_start(
                            out=xcs[c], in_=xT[:, c * CH : (c + 1) * CH]
                        )
                xc = xcs[c]
                xa = xc[:, ci * B : (ci + 1) * B]
                xb = xc[:, (ci + 1) * B : (ci + 2) * B]

                # ---- layer 1 ----
                pa1a = paP.tile([128, B], F32, tag="pa", name="pa1a")
                nc.tensor.matmul(pa1a, W1s, xa)
                pa1b = paP.tile([128, B], F32, tag="pa", name="pa1b")
                nc.tensor.matmul(pa1b, W1s, xb)

                h1w = l1p.tile([128, 2 * B], F16, tag="h1w", name="h1w")
                nc.scalar.activation(out=h1w[:, 0:B], in_=pa1a, func=Tanh, bias=b1s)
                nc.scalar.activation(
                    out=h1w[:, B : 2 * B], in_=pa1b, func=Tanh, bias=b1s
                )
                hh1w = l1p.tile([128, 2 * B], F16, tag="hh1w", name="hh1w")
                nc.gpsimd.tensor_mul(
                    out=hh1w[:, 0:B], in0=h1w[:, 0:B], in1=h1w[:, 0:B]
                )
                nc.gpsimd.tensor_mul(
                    out=hh1w[:, B : 2 * B], in0=h1w[:, B : 2 * B],
                    in1=h1w[:, B : 2 * B],
                )
                # mg1 = hh1 - 1 = -g1  (a-half ACT Copy, b-half DVE TS)
                mg1w = l1p.tile([128, 2 * B], F16, tag="mg1w", name="mg1w")
                nc.scalar.activation(
                    out=mg1w[:, 0:B], in_=hh1w[:, 0:B],
                    func=mybir.ActivationFunctionType.Copy, bias=-1.0,
                )
                nc.vector.tensor_scalar(
                    out=mg1w[:, B : 2 * B], in0=hh1w[:, B : 2 * B],
                    scalar1=1.0, scalar2=None, op0=sub,
                )
                # u1n = h1 * mg1 = -h1*g1  (TT-1024)
                u1nw = l1p.tile([128, 2 * B], F16, tag="u1nw", name="u1nw")
                nc.vector.tensor_mul(out=u1nw, in0=h1w, in1=mg1w)

                # ---- layer 2 matmuls ----
                pa2a = paP.tile([128, B], F32, tag="pa", name="pa2a")
                nc.tensor.matmul(pa2a, W2s, h1w[:, 0:B])
                pa2b = paP.tile([128, B], F32, tag="pa", name="pa2b")
                nc.tensor.matmul(pa2b, W2s, h1w[:, B : 2 * B])
                dz2a = dztP.tile([128, 2 * B], F32, tag="dzt", name="dz2a")
                nc.tensor.matmul(dz2a[:, 0:B], Z2s, mg1w[:, 0:B])
                nc.tensor.matmul(dz2a[:, B : 2 * B], T2s, mg1w[:, 0:B])
                dz2b = dztP.tile([128, 2 * B], F32, tag="dzt", name="dz2b")
                nc.tensor.matmul(dz2b[:, 0:B], Z2s, mg1w[:, B : 2 * B])
                nc.tensor.matmul(dz2b[:, B : 2 * B], T2s, mg1w[:, B : 2 * B])
                # app2 = ZZ2^T u1n = a2''  (accumulates -v2 later)
                app2a = appP.tile([128, B], F32, tag="app", name="app2a")
                nc.tensor.matmul(app2a, ZZ2s, u1nw[:, 0:B], start=True, stop=False)
                app2b = appP.tile([128, B], F32, tag="app", name="app2b")
                nc.tensor.matmul(
                    app2b, ZZ2s, u1nw[:, B : 2 * B], start=True, stop=False
                )

                # ---- layer 2 pointwise ----
                h2w = l2p.tile([128, 2 * B], F16, tag="h2w", name="h2w")
                nc.scalar.activation(out=h2w[:, 0:B], in_=pa2a, func=Tanh, bias=b2s)
                nc.scalar.activation(
                    out=h2w[:, B : 2 * B], in_=pa2b, func=Tanh, bias=b2s
                )
                hh2w = l2p.tile([128, 2 * B], F16, tag="hh2w", name="hh2w")
                nc.vector.tensor_mul(out=hh2w, in0=h2w, in1=h2w)
                sq2w = l2p.tile([128, 2 * B], F16, tag="sq2w", name="sq2w")
                nc.scalar.activation(out=sq2w[:, 0:B], in_=dz2a[:, 0:B], func=Square)
                nc.scalar.activation(
                    out=sq2w[:, B : 2 * B], in_=dz2b[:, 0:B], func=Square
                )
                v2w = l2p.tile([128, 2 * B], F16, tag="v2w", name="v2w")
                nc.gpsimd.tensor_mul(
                    out=v2w[:, 0:B], in0=h2w[:, 0:B], in1=sq2w[:, 0:B]
                )
                nc.gpsimd.tensor_mul(
                    out=v2w[:, B : 2 * B], in0=h2w[:, B : 2 * B],
                    in1=sq2w[:, B : 2 * B],
                )
                # mzt2 = (hh2-1)*(az|at): ztw = [tz_a|tz_b|tt_a|tt_b]
                ztw = l2p.tile([128, 4 * B], F16, tag="ztw", name="ztw")
                stt(
                    nc.vector,
                    split2(
                        bass.AP(tensor=ztw.tensor, offset=ztw[:].offset,
                                ap=ztw[:].ap),
                        2 * B,
                    ),
                    bc2(hh2w[:, 0:B]),
                    dz2a[:],
                )
                stt(
                    nc.vector,
                    split2(
                        bass.AP(tensor=ztw.tensor, offset=ztw[:].offset + B,
                                ap=ztw[:].ap),
                        2 * B,
                    ),
                    bc2(hh2w[:, B : 2 * B]),
                    dz2b[:],
                )
                # i2 = app2 - v2 (PE accumulate)
                nc.tensor.matmul(app2a, negIs, v2w[:, 0:B], start=False, stop=True)
                nc.tensor.matmul(
                    app2b, negIs, v2w[:, B : 2 * B], start=False, stop=True
                )
                # mhpp2 = (hh2-1)*i2 = -h2''
                mhpp2 = l2p.tile([128, 2 * B], F16, tag="mhpp2", name="mhpp2")
                stt(nc.vector, mhpp2[:, 0:B], hh2w[:, 0:B], app2a[:])
                stt(nc.vector, mhpp2[:, B : 2 * B], hh2w[:, B : 2 * B], app2b[:])
                return {"ta": ta, "h2w": h2w, "ztw": ztw, "mhpp2": mhpp2}

            def stage2(s):
                """L3 + L4 for pair described by state s."""
                ta, h2w, ztw, mhpp2 = s["ta"], s["h2w"], s["ztw"], s["mhpp2"]
                tz2a = ztw[:, 0:B]
                tz2b = ztw[:, B : 2 * B]
                tt2a = ztw[:, 2 * B : 3 * B]
                tt2b = ztw[:, 3 * B : 4 * B]
                pa3 = paP.tile([128, B], F32, tag="pa", name="pa3")
                nc.tensor.matmul(pa3[0:64], W3s, h2w[:, 0:B])
                nc.tensor.matmul(pa3[64:128], W3s, h2w[:, B : 2 * B])
                dz3 = dztP.tile([128, 2 * B], F32, tag="dzt", name="dz3")
                nc.tensor.matmul(dz3[0:64, 0:B], W3s, tz2a)  # sqrt2*a3z'
                nc.tensor.matmul(dz3[64:128, 0:B], W3s, tz2b)
                nc.tensor.matmul(dz3[0:64, B : 2 * B], W3s, tt2a)  # a3t'
                nc.tensor.matmul(dz3[64:128, B : 2 * B], W3s, tt2b)
                papp3 = appP.tile([128, B], F32, tag="app", name="papp3")
                nc.tensor.matmul(
                    papp3[0:64], W3ns, mhpp2[:, 0:B], start=True, stop=False
                )
                nc.tensor.matmul(
                    papp3[64:128], W3ns, mhpp2[:, B : 2 * B], start=True, stop=False
                )

                # ---- layer 3 pointwise ----
                h3t = l3p.tile([128, B], F16, tag="h3", name="h3")
                h3 = h3t[:]
                nc.scalar.activation(out=h3, in_=pa3, func=Tanh, bias=b3ds)
                hh3t = l3p.tile([128, B], F16, tag="hh3", name="hh3")
                hh3 = hh3t[:]
                nc.scalar.activation(out=hh3, in_=h3, func=Square)
                sq3 = l3p.tile([128, B], F16, tag="sq3", name="sq3")
                nc.scalar.activation(out=sq3, in_=dz3[:, 0:B], func=Square)
                v3 = l3p.tile([128, B], F16, tag="v3", name="v3")
                nc.gpsimd.tensor_mul(out=v3, in0=h3, in1=sq3)
                zt3 = l3p.tile([128, 2 * B], F16, tag="zt3", name="zt3")
                stt(nc.vector, zt3[:], bc2(hh3), dz3[:])
                nc.tensor.matmul(papp3, negIs, v3, start=False, stop=True)
                mhpp3 = l3p.tile([128, B], F16, tag="mhpp3", name="mhpp3")
                stt(nc.vector, mhpp3, hh3, papp3[:])

                # ---- layer 4 ----
                p4 = appP.tile([8, B], F32, tag="app", name="p4")
                nc.tensor.matmul(p4, W4hs, h3, start=True, stop=False)
                nc.tensor.matmul(p4, W4zs, zt3[:, 0:B], start=False, stop=False)
                nc.tensor.matmul(p4, W4ts, zt3[:, B : 2 * B], start=False, stop=False)
                nc.tensor.matmul(p4, W4ps, mhpp3, start=False, stop=True)
                with tc.high_priority(offset=30):
                    sb4 = sb4p.tile([8, B], F32, tag="sb4", name="sb4")
                    nc.scalar.copy(out=sb4, in_=p4)
                ofull = out_d[:]
                o8 = bass.AP(
                    tensor=ofull.tensor,
                    offset=ofull.offset + ta * B,
                    ap=[[B, 2], [NSH, 4], [1, B]],
                )
                nc.sync.dma_start(out=o8, in_=sb4)

            for p in range(NP):
                stage2(stage1(p))

    nc.finalize()
    return nc


_NC_CACHE = None


def _get_nc():
    global _NC_CACHE
    if _NC_CACHE is None:
        _NC_CACHE = _build()
    return _NC_CACHE


def kernel(**inputs):
    global LAST_RESULT
    f = np.float32
    f16 = np.float16
    x = np.asarray(inputs["x"], dtype=f)
    W1 = np.asarray(inputs["W1"], dtype=f)
    b1 = np.asarray(inputs["b1"], dtype=f)
    W2 = np.asarray(inputs["W2"], dtype=f)
    b2 = np.asarray(inputs["b2"], dtype=f)
    W3 = np.asarray(inputs["W3"], dtype=f)
    b3 = np.asarray(inputs["b3"], dtype=f)
    W4 = np.asarray(inputs["W4"], dtype=f)
    b4 = np.asarray(inputs["b4"], dtype=f)

    xT = np.ascontiguousarray(x.T)  # [3, N]
    w4 = W4[:, 0].astype(f)
    SQ2 = np.sqrt(2.0).astype(f)

    W4h = np.zeros((128, 8), f)
    W4h[0:64, 0] = w4
    W4h[64:128, 4] = w4
    W4z = np.zeros((128, 8), f)
    W4z[0:64, 1] = -w4 / SQ2
    W4z[64:128, 5] = -w4 / SQ2
    W4t = np.zeros((128, 8), f)
    W4t[0:64, 2] = -w4
    W4t[64:128, 6] = -w4
    W4p = np.zeros((128, 8), f)
    W4p[0:64, 3] = -w4
    W4p[64:128, 7] = -w4

    wpk = np.concatenate(
        [
            W2,
            SQ2 * W1[0][:, None] * W2,
            W1[1][:, None] * W2,
            2.0 * (W1[0] ** 2)[:, None] * W2,
            W3,
            -W3,
            W4h,
            W4z,
            W4t,
            W4p,
            -np.eye(128, dtype=f),
        ],
        axis=1,
    ).astype(f16)
    bpk = np.stack(
        [b1, b2, np.concatenate([b3, b3]), np.full(128, -1.0)], axis=1
    ).astype(f)  # [128, 4]
    common = {
        "W1": W1.astype(f16),
        "Wpk": np.ascontiguousarray(wpk),
        "bpk": np.ascontiguousarray(bpk),
    }
    in_maps = [
        dict(
            common,
            xT=np.ascontiguousarray(xT[:, i * NSH : (i + 1) * NSH]).astype(f16),
        )
        for i in range(NCORES)
    ]

    nc = _get_nc()
    res = run_bass_kernel_spmd(nc, in_maps, list(range(NCORES)), trace=TRACE)
    LAST_RESULT = res

    full = np.concatenate(
        [res.results[i]["out"] for i in range(NCORES)], axis=1
    )  # [4, N] rows (T, Tz, Tt, Tpp)
    out = np.ascontiguousarray(full.T).astype(f)
    out[:, 0] += b4[0]
    return out
